# revision 6
# baseline (speedup 1.0000x reference)
"""Trainium2 Bass kernel for nn_DARPDecoder (sparse_attention).

Pure data parallel over batch: 8 cores x 128 batches. Per batch:
  score[b,n] = emb[b,n,:].qk[b] - C*T[cur_h3[b], h3[b,n]]; tanh-clip, mask,
  log_softmax, where qk[b] = sum_i (W_i @ W_key^T/sqrt(D))^T h_i[b] — W_key and
  the 1/sqrt(D) are folded into the five projection weights on the host, which
  removes the [B,N,D] K tensor and a whole matmul stage.

The end-to-end metric is dominated by host->device input transfer over the
axon tunnel (~50 MB/s, uncompressible data), so the kernel is organized to
minimize bytes shipped per core (~17.3 MB, vs 35.5 MB for the two-layout
variant):
  - node_emb ships ONCE per core, bf16, in the natural [n,d] chunk layout.
    The [d,n] tiles needed by pass 2 are produced on device by PE transposes
    (32 x [128,128] per chunk) into a rotating PSUM pool, copied to SBUF by
    DVE. PE/DVE time is microseconds; transfer savings are seconds.
  - travel_time_matrix is not shipped; the host gathers the 128 needed rows
    (T[cur_h3[b], :]) and pre-scales by C_TRAVEL into a bf16 [128,512] input.
  - the 16 psel selection matrices (-1 diagonals on rows {16g+k}) are built
    on device from the identity and a [128,16] mask table (4 KB) instead of
    shipping 512 KB of constants.
  - visited/action_mask ship as one u8 [128,512] with 2 bits packed
    (vis + 2*act), unpacked by 4 DVE ops.

Device structure (unchanged math from the validated two-layout kernel):
  - pass 1: lhsT = nat quarter [128n,128d], rhs = [ones/512 | vf/cnt] 2 cols
    -> graph/visited sums, PSUM [128d, 2 cols/batch], pool-rotated tiles.
  - pass 2: lhsT = transposed quarter [128d,128n], rhs = qk[:,b] single col
    -> score quarters, copied per-chunk to SBUF, transposed back to
    [128b, 512n] by 4 TensorE transposes at the tail.
  - travel bias: per-batch gpsimd indirect_copy from the host-gathered rows
    (each 16-partition group shares one index stream; psel[k] keeps exactly
    rows {16g+k}), accumulated into one PSUM bank during the stream.
  - tail: tanh -> exp(10*tanh) on Act, masked softmax-sum via Pool-multiply +
    DVE-reduce; log_softmax needs no max shift (tanh clips |s| to 10).

Known-invalid-on-HW constructs (sim passes, device faults) to avoid:
  tensor_tensor_reduce; concurrent PE-write + DVE-read on one PSUM bank;
  two PSUM operands in one DVE op.
"""

import functools
import math
from concurrent.futures import ThreadPoolExecutor

import numpy as np
import ml_dtypes

import concourse.bass as bass
import concourse.mybir as mybir
import concourse.tile as tile
from concourse import bacc
from concourse.bass_utils import run_bass_kernel_spmd

BF16 = mybir.dt.bfloat16
F32 = mybir.dt.float32
I32 = mybir.dt.int32
U16 = mybir.dt.uint16
U8 = mybir.dt.uint8
Alu = mybir.AluOpType
AF = mybir.ActivationFunctionType
AX = mybir.AxisListType

B, N, D, NCORES = 1024, 512, 128, 8
BC = B // NCORES  # 128 batches/core
NCH, CHB = 16, 8  # 16 stream chunks x 8 batches
MAX_TIME = 1440.0
TANH_CLIP = 10.0
C_TRAVEL = 1.0 / MAX_TIME / math.sqrt(2.0)
INV_SQRT_D = 1.0 / math.sqrt(D)
NBF = np.dtype(ml_dtypes.bfloat16)

# consts layout: [idn 128 | wl 128 | wf 128 | wg 128 | wv 128 | ws 128]
CW_COLS = 128 + 5 * 128

# single packed input blob: one u16 [R0_TOT, 128] tensor per core. Each
# region's logical [128, X]-shaped view occupies X//128 consecutive DRAM rows
# per logical row (plain row-major reshape), bitcast to its real dtype.
R0_EMB = 0                       # [65536, 128] bf16 (nat chunk layout)
R0_H3W = R0_EMB + NCH * 128 * 32  # [128, 512] u16
R0_RBF = R0_H3W + 512            # [128, 512] bf16
R0_VAM = R0_RBF + 512            # [128, 512] u8
R0_CW = R0_VAM + 256             # [128, 768] bf16
R0_MISC = R0_CW + 768            # [128, 128] u16: smallf f32 | smalli i32 | negm f32
R0_TOT = R0_MISC + 128

# queue for each streamed nat chunk. gpsimd (Pool software-DGE) takes the
# first 4 back-to-back (its indirect_copy work fills the later window);
# sync/scalar alternate over the rest.
NAT_Q = ["gpsimd"] * 4 + ["sync", "scalar"] * 6


def _emit(nc, tc, T):
    blob = T["blob"].ap()
    ap = {
        "emb_nat_t": blob[R0_EMB : R0_EMB + 65536, :].bitcast(BF16),
        "h3w": blob[R0_H3W : R0_H3W + 512, :].rearrange(
            "(p f) c -> p (f c)", f=4),
        "rbf": blob[R0_RBF : R0_RBF + 512, :].bitcast(BF16).rearrange(
            "(p f) c -> p (f c)", f=4),
        "vam": blob[R0_VAM : R0_VAM + 256, :].bitcast(U8).rearrange(
            "(p t) c -> p (t c)", t=2),
        "constsW": blob[R0_CW : R0_CW + 768, :].bitcast(BF16).rearrange(
            "(p s) c -> p (s c)", s=6),
        "smallf": blob[R0_MISC : R0_MISC + 128, 0:8].bitcast(F32),
        "smalli": blob[R0_MISC : R0_MISC + 128, 8:16].bitcast(I32),
        "negm": blob[R0_MISC : R0_MISC + 128, 16:48].bitcast(F32),
        "out": T["out"].ap(),
    }
    eng = {"sync": nc.sync, "scalar": nc.scalar, "gpsimd": nc.gpsimd}

    with (
        tc.tile_pool(name="cp", bufs=1) as cp,
        tc.tile_pool(name="stn", bufs=6) as stn,
        tc.tile_pool(name="ste", bufs=3) as ste,
        tc.tile_pool(name="wk", bufs=2) as wk,
        tc.tile_pool(name="ps_s", bufs=2, space="PSUM") as ps_s,
        tc.tile_pool(name="ps_q", bufs=2, space="PSUM") as ps_q,
        tc.tile_pool(name="ps_tv", bufs=1, space="PSUM") as ps_tv,
        tc.tile_pool(name="ps_e", bufs=2, space="PSUM") as ps_e,
        tc.tile_pool(name="ps_qk", bufs=1, space="PSUM") as ps_qk,
    ):
        # ---------- consolidated small loads ----------
        # scalar queue: consts first (idn gates all prologue transposes),
        # then the travel rows and packed masks, then its nat tiles.
        cw = cp.tile([128, CW_COLS], BF16, name="constsW")
        nc.scalar.dma_start(out=cw[:], in_=ap["constsW"])
        rbf = cp.tile([BC, N], BF16, name="rbf")
        nc.scalar.dma_start(out=rbf[:], in_=ap["rbf"])
        vam = cp.tile([BC, N], U8, name="vam")
        nc.scalar.dma_start(out=vam[:], in_=ap["vam"])
        smf = cp.tile([128, 4], F32, name="smallf")
        nc.sync.dma_start(out=smf[:], in_=ap["smallf"])
        smi = cp.tile([128, 4], I32, name="smalli")
        nc.sync.dma_start(out=smi[:], in_=ap["smalli"])
        gcur = cp.tile([BC, 1], I32, name="gcur")
        nc.vector.tensor_copy(out=gcur[:], in_=smi[:, 0:1])
        gfn = cp.tile([BC, 1], I32, name="gfn")
        nc.vector.tensor_copy(out=gfn[:], in_=smi[:, 1:2])
        gcur, gfn = gcur[:], gfn[:]
        h3w = cp.tile([128, 512], U16, name="h3w")
        nc.gpsimd.dma_start(out=h3w[:], in_=ap["h3w"])

        idn = cw[:, 0:128]
        wb = 128
        wl, wf, wg, wv = (cw[:, wb + 128 * i : wb + 128 * (i + 1)] for i in range(4))
        ws = cw[0:4, wb + 512 : wb + 640]

        # ---------- activation-table preload (tanh+exp share one table) ----------
        dum = cp.tile([1, 1], F32, name="dum")
        nc.vector.memset(dum[:], 1.0)
        dmo = wk.tile([1, 1], F32, tag="dmo")
        nc.scalar.activation(out=dmo[:], in_=dum[:], func=AF.Tanh, scale=1.0)

        idnf = cp.tile([128, 128], F32, name="idnf")
        nc.vector.tensor_copy(out=idnf[:], in_=idn)

        # psel[k] = idn * negm[:,k] (per-partition broadcast): -1 diagonal on
        # rows {16g+k}, zero elsewhere. (scalar1 APs must be f32.)
        negmf = cp.tile([128, 16], F32, name="negmf")
        nc.sync.dma_start(out=negmf[:], in_=ap["negm"])
        pselt = cp.tile([128, 16 * 128], BF16, name="pselt")
        for k in range(16):
            nc.vector.tensor_scalar(
                out=pselt[:, 128 * k : 128 * (k + 1)], in0=idn,
                scalar1=negmf[:, k : k + 1], scalar2=None, op0=Alu.mult)
        psel = [pselt[:, 128 * k : 128 * (k + 1)] for k in range(16)]

        # ---------- gathers (gpsimd queue; need smalli only) ----------
        hc_rows = cp.tile([BC, D], BF16, name="hc_rows")
        nc.gpsimd.indirect_dma_start(
            out=hc_rows[:], out_offset=None, in_=ap["emb_nat_t"],
            in_offset=bass.IndirectOffsetOnAxis(ap=gcur, axis=0))
        hf_rows = cp.tile([BC, D], BF16, name="hf_rows")
        nc.gpsimd.indirect_dma_start(
            out=hf_rows[:], out_offset=None, in_=ap["emb_nat_t"],
            in_offset=bass.IndirectOffsetOnAxis(ap=gfn, axis=0))

        # ---------- masks / counts (vam = visited + 2*action_mask) ----------
        vamf = cp.tile([BC, N], F32, name="vamf")
        nc.vector.tensor_copy(out=vamf[:], in_=vam[:])
        amf = cp.tile([BC, N], F32, name="amf")
        nc.vector.tensor_scalar(out=amf[:], in0=vamf[:], scalar1=2.0,
                                scalar2=None, op0=Alu.is_ge)
        amn = cp.tile([BC, N], F32, name="amn")
        nc.vector.tensor_scalar_mul(out=amn[:], in0=amf[:], scalar1=-2.0)
        visf = cp.tile([BC, N], F32, name="visf")
        nc.vector.tensor_add(out=visf[:], in0=vamf[:], in1=amn[:])
        vc = cp.tile([BC, 1], F32, name="vc")
        nc.vector.tensor_reduce(out=vc[:], in_=visf[:], axis=AX.X, op=Alu.add)
        nc.vector.tensor_scalar_max(out=vc[:], in0=vc[:], scalar1=1.0)
        vcr = cp.tile([BC, 1], F32, name="vcr")
        nc.vector.reciprocal(out=vcr[:], in_=vc[:])
        vsc = cp.tile([BC, N], BF16, name="vsc")
        nc.vector.tensor_scalar(out=vsc[:], in0=visf[:], scalar1=vcr[:, :1],
                                scalar2=None, op0=Alu.mult)

        # vs2[q]: [128 n_q, 2*BC] cols (2b, 2b+1) = (1/512, vf[b, n_q]/cnt_b)
        vs2 = []
        for q in range(4):
            v = cp.tile([128, 2 * BC], BF16, name=f"vs2_{q}")
            nc.vector.memset(v[:].rearrange("p (b two) -> p b two", two=2)[:, :, 0:1],
                             1.0 / N)
            pt = ps_e.tile([128, 512], BF16, tag="et_ps")
            nc.tensor.transpose(out=pt[:, 0:128], in_=vsc[:, 128 * q : 128 * (q + 1)],
                                identity=idn)
            nc.vector.tensor_copy(
                out=v[:].rearrange("p (b two) -> p b two", two=2)[:, :, 1:2],
                in_=pt[:, 0:128].rearrange("p (b one) -> p b one", one=1))
            vs2.append(v)

        # ---------- masks for the epilogue (hoisted off the tail) ----------
        m10 = cp.tile([BC, N], F32, name="m10")
        nc.vector.tensor_scalar_mul(out=m10[:], in0=amf[:], scalar1=TANH_CLIP)
        m2 = cp.tile([BC, N], F32, name="m2")
        nc.vector.tensor_scalar(out=m2[:], in0=amf[:], scalar1=1.0, scalar2=1e8,
                                op0=Alu.subtract, op1=Alu.mult)

        # ---------- h_cur/h_first/state transposes ----------
        hct = cp.tile([128, BC], BF16, name="hct")
        pt1 = ps_e.tile([128, 512], BF16, tag="et_ps")
        nc.tensor.transpose(out=pt1[:, 0:128], in_=hc_rows[:], identity=idn)
        nc.vector.tensor_copy(out=hct[:], in_=pt1[:, 0:128])
        hft = cp.tile([128, BC], BF16, name="hft")
        pt2 = ps_e.tile([128, 512], BF16, tag="et_ps")
        nc.tensor.transpose(out=pt2[:, 0:128], in_=hf_rows[:], identity=idn)
        nc.vector.tensor_copy(out=hft[:], in_=pt2[:, 0:128])

        sf = cp.tile([BC, 4], F32, name="sf")
        nc.vector.tensor_sub(out=sf[:, 0:1], in0=smf[:, 2:3], in1=smf[:, 1:2])
        nc.vector.tensor_scalar_mul(out=sf[:, 1:2], in0=smf[:, 0:1], scalar1=1.0 / MAX_TIME)
        nc.vector.tensor_scalar_mul(out=sf[:, 2:3], in0=smf[:, 3:4], scalar1=1.0 / (2.0 * N))
        nc.vector.memset(sf[:, 3:4], 1.0)
        sfb = cp.tile([BC, 4], BF16, name="sfb")
        nc.vector.tensor_copy(out=sfb[:], in_=sf[:])
        pt3 = ps_e.tile([128, 512], BF16, tag="et_ps")
        nc.tensor.transpose(out=pt3[:4, 0:128], in_=sfb[:], identity=idn)
        sft = cp.tile([4, BC], BF16, name="sft")
        nc.vector.tensor_copy(out=sft[:], in_=pt3[:4, :BC])

        # ---------- persistent accumulators ----------
        gvb = cp.tile([128, 2 * BC], BF16, name="gvb")
        qk = cp.tile([128, BC], BF16, name="qk")
        scA = cp.tile([128, N], F32, name="scA")           # scores [n_q, (q,b)]
        pvT = ps_tv.tile([128, N], F32, tag="trav")        # -C*travel (PE-only)

        nat_src = ap["emb_nat_t"].rearrange("(k p blk) d -> k p (blk d)",
                                            k=NCH, p=128, blk=32)

        # travel steps spread over stream iterations: gather gk[t] rows
        # {16g+t} = C*T[cur_h3[b], h3[b,:]] straight from rbf (rows of other
        # batches in each 16-partition group are garbage; psel[t] keeps only
        # row 16g+t), then accumulate -travel into pvT.
        ic_iter = [1 + (k * 12) // 16 for k in range(16)]
        mm_iter = [min(NCH - 2, i + 1) for i in ic_iter]
        gk = [None] * 16

        # ---------- streamed chunks ----------
        for k in range(NCH):
            nat = stn.tile([128, 4096], BF16, tag="nat")
            eng[NAT_Q[k]].dma_start(out=nat[:], in_=nat_src[k])

            for t in [i for i, it in enumerate(ic_iter) if it == k]:
                g = cp.tile([128, N], BF16, name=f"gk{t}")
                nc.gpsimd.indirect_copy(out=g[:], data=rbf[:],
                                        idxs=h3w[:, 32 * t : 32 * (t + 1)],
                                        i_know_ap_gather_is_preferred=True)
                gk[t] = g
            for t in [i for i, it in enumerate(mm_iter) if it == k]:
                nc.tensor.matmul(out=pvT[:], lhsT=psel[t], rhs=gk[t][:],
                                 start=(t == 0), stop=True, skip_group_check=True)
            if k == NCH - 1:
                tvs = cp.tile([BC, N], F32, name="tvs")
                nc.vector.tensor_copy(out=tvs[:], in_=pvT[:])

            # build the [d,n] chunk by PE transposes of the 32 nat blocks
            et = ste.tile([128, 4096], BF16, tag="et")
            for j in range(CHB):
                pe = ps_e.tile([128, 512], BF16, tag="et_ps")
                for q in range(4):
                    nc.tensor.transpose(
                        out=pe[:, 128 * q : 128 * (q + 1)],
                        in_=nat[:, (j * 4 + q) * 128 : (j * 4 + q + 1) * 128],
                        identity=idn)
                nc.vector.tensor_copy(out=et[:, j * 512 : (j + 1) * 512], in_=pe[:])

            # pass 1: graph/visited sums, batch j -> pSk[:, 2j:2j+2]
            pSk = ps_s.tile([128, 2 * CHB], F32, tag="sums")
            for j in range(CHB):
                b = k * CHB + j
                for q in range(4):
                    nc.tensor.matmul(
                        out=pSk[:, 2 * j : 2 * j + 2],
                        lhsT=nat[:, (j * 4 + q) * 128 : (j * 4 + q + 1) * 128],
                        rhs=vs2[q][:, 2 * b : 2 * b + 2],
                        start=(q == 0), stop=(q == 3), skip_group_check=True)
            nc.vector.tensor_copy(out=gvb[:, 16 * k : 16 * (k + 1)], in_=pSk[:])

            # q/qk for this chunk's 8 batches
            sl = slice(8 * k, 8 * (k + 1))
            g_sl = gvb[:, 16 * k : 16 * (k + 1)].rearrange("p (b two) -> p b two", two=2)
            psq = ps_qk.tile([128, 8], F32, tag="psq")
            nc.tensor.matmul(out=psq[:], lhsT=wl, rhs=hct[:, sl], start=True, stop=True)
            nc.tensor.matmul(out=psq[:], lhsT=wf, rhs=hft[:, sl], start=False, stop=True,
                             skip_group_check=True)
            nc.tensor.matmul(out=psq[:], lhsT=wg, rhs=g_sl[:, :, 0:1], start=False,
                             stop=True, skip_group_check=True)
            nc.tensor.matmul(out=psq[:], lhsT=wv, rhs=g_sl[:, :, 1:2], start=False,
                             stop=True, skip_group_check=True)
            nc.tensor.matmul(out=psq[:], lhsT=ws, rhs=sft[:, sl], start=False, stop=True,
                             skip_group_check=True)
            nc.vector.tensor_copy(out=qk[:, sl], in_=psq[:])

            # pass 2: score quarters, batch j -> pqk[:, 4j+q]; copy to the
            # quarter-major SBUF tile right away (keeps the tail short)
            pqk = ps_q.tile([128, 4 * CHB], F32, tag="scT")
            for j in range(CHB):
                b = k * CHB + j
                for q in range(4):
                    nc.tensor.matmul(
                        out=pqk[:, 4 * j + q : 4 * j + q + 1],
                        lhsT=et[:, j * 512 + 128 * q : j * 512 + 128 * (q + 1)],
                        rhs=qk[:, b : b + 1],
                        start=True, stop=True, skip_group_check=True)
            nc.vector.tensor_copy(
                out=scA[:].rearrange("p (q b) -> p q b", q=4)[:, :, 8 * k : 8 * (k + 1)],
                in_=pqk[:].rearrange("p (b q) -> p q b", q=4))

        # ---------- tail: per-half pipeline across PE/Act/DVE ----------
        # scA holds all scores [n_q, 4b+q]; transpose quarter q via stride-4
        # lhsT view, accumulating onto -travel in pvT. Then per half:
        # th = tanh(s/10) (Act), exm = exp(10*th) (Act, no mask needed first),
        # se = sum(exm*am) fused on DVE, msk for the output in parallel.
        # log_softmax has no max shift (tanh clips |s| to 10).
        msk = cp.tile([BC, N], F32, name="msk")
        seq = cp.tile([BC, 2], F32, name="seq")
        # transpose the four score quarters into pvT's bank (free after the tvs
        # copy; all 4 PE writes precede the single DVE read -> no bank overlap)
        ssb = cp.tile([BC, N], F32, name="ssb")
        for qq in range(4):
            qb = slice(128 * qq, 128 * (qq + 1))
            nc.tensor.transpose(out=pvT[:, qb], in_=scA[:, qb], identity=idnf[:])
        nc.vector.tensor_add(out=ssb[:], in0=pvT[:], in1=tvs[:])
        for h in range(2):
            blk = slice(256 * h, 256 * (h + 1))
            thq = wk.tile([128, 256], F32, tag="thq")
            nc.scalar.activation(out=thq[:], in_=ssb[:, blk], func=AF.Tanh,
                                 scale=1.0 / TANH_CLIP)
            exq = wk.tile([128, 256], F32, tag="exq")
            nc.scalar.activation(out=exq[:], in_=thq[:], func=AF.Exp,
                                 scale=TANH_CLIP)
            exm = wk.tile([128, 256], F32, tag="exm")
            nc.gpsimd.tensor_mul(out=exm[:], in0=exq[:], in1=amf[:, blk])
            nc.vector.tensor_reduce(out=seq[:, h : h + 1], in_=exm[:], axis=AX.X,
                                    op=Alu.add)
            nc.gpsimd.tensor_mul(out=msk[:, blk], in0=thq[:], in1=m10[:, blk])
            nc.gpsimd.tensor_add(out=msk[:, blk], in0=msk[:, blk], in1=m2[:, blk])
        se = cp.tile([BC, 1], F32, name="se")
        nc.gpsimd.tensor_add(out=se[:], in0=seq[:, 0:1], in1=seq[:, 1:2])
        lse = cp.tile([BC, 1], F32, name="lse")
        nc.scalar.activation(out=lse[:], in_=se[:], func=AF.Ln)
        fin = wk.tile([BC, N], BF16, tag="fin")
        for h, ve, de in ((0, nc.vector, nc.sync), (1, nc.gpsimd, nc.scalar)):
            blk = slice(256 * h, 256 * (h + 1))
            ve.tensor_scalar(out=fin[:, blk], in0=msk[:, blk],
                             scalar1=lse[:, :1], scalar2=None, op0=Alu.subtract)
            de.dma_start(out=ap["out"][:, blk], in_=fin[:, blk])


def build_program():
    nc = bacc.Bacc("TRN2", target_bir_lowering=False, debug=False)
    dt = nc.dram_tensor
    T = {}

    def din(name, shape, dtype):
        T[name] = dt(name, shape, dtype, kind="ExternalInput")

    din("blob", [R0_TOT, 128], U16)  # all inputs, packed (see R0_* layout)
    T["out"] = dt("out", [BC, N], BF16, kind="ExternalOutput")

    with tile.TileContext(nc) as tc:
        _emit(nc, tc, T)
    nc.compile()
    return nc


@functools.cache
def _cached_program():
    return build_program()


def _consts():
    negm = np.zeros((128, 16), np.float32)
    p = np.arange(128)
    for k in range(16):
        negm[p % 16 == k, k] = -1.0
    return {"_idn": np.eye(128, dtype=NBF), "_negm": negm}


def make_in_map(inputs, core, consts=None, embb_all=None):
    """Host-side shard + relayout for one core (pure layout/dtype work)."""
    sl = slice(BC * core, BC * (core + 1))
    if embb_all is not None:
        embb = embb_all[sl]
    else:
        embb = np.asarray(inputs["node_emb"][sl], dtype=np.float32).astype(NBF)
    consts = consts or _consts()
    blob = np.zeros((R0_TOT, 128), np.uint16)
    # emb in nat chunk layout, relayouted straight into the blob
    np.copyto(blob[R0_EMB : R0_EMB + 65536].view(NBF).reshape(NCH, 128, CHB, 4, D),
              embb.reshape(NCH, CHB, 4, 128, D).transpose(0, 3, 1, 2, 4))
    h3 = np.asarray(inputs["h3_indices"][sl]).astype(np.int32)
    h3w = np.ascontiguousarray(
        h3.reshape(8, 16, 32, 16).transpose(1, 0, 3, 2).reshape(16, 128, 32)
        .transpose(1, 0, 2)).reshape(128, 512).astype(np.uint16)
    blob[R0_H3W : R0_H3W + 512] = h3w.reshape(512, 128)
    vam = (np.asarray(inputs["visited"][sl]).astype(np.uint8)
           + 2 * np.asarray(inputs["action_mask"][sl]).astype(np.uint8))
    blob[R0_VAM : R0_VAM + 256] = vam.view(np.uint16).reshape(256, 128)
    wkT = np.asarray(inputs["W_key"], np.float32).T * INV_SQRT_D
    w = np.zeros((128, 640), np.float32)
    w[:, 0:128] = np.asarray(inputs["W_last"], np.float32) @ wkT
    w[:, 128:256] = np.asarray(inputs["W_first"], np.float32) @ wkT
    w[:, 256:384] = np.asarray(inputs["W_graph"], np.float32) @ wkT
    w[:, 384:512] = np.asarray(inputs["W_visited"], np.float32) @ wkT
    w[0:3, 512:640] = np.asarray(inputs["W_state"], np.float32) @ wkT
    w[3, 512:640] = np.asarray(inputs["b_state"], np.float32) @ wkT
    cwv = blob[R0_CW : R0_CW + 768].view(NBF).reshape(128, 768)
    cwv[:, 0:128] = consts["_idn"]
    cwv[:, 128:768] = w.astype(NBF)
    cur = np.asarray(inputs["current_node"][sl]).astype(np.int64)[:, 0]
    prv = np.asarray(inputs["previous_action"][sl]).astype(np.int64)[:, 0]
    fst = np.asarray(inputs["first_node"][sl]).astype(np.int64)
    fn = np.where((prv == 0) & (cur != 0), cur, fst)
    fn = np.where(cur == 0, 0, fn)
    bb = np.arange(BC)

    def nat_row(idx):
        # row of emb_nat_t [(k, n%128, (j,q))] holding emb[b, idx, :]
        return (bb // 8) * 4096 + (idx % 128) * 32 + (bb % 8) * 4 + idx // 128

    misc = blob[R0_MISC : R0_MISC + 128]
    sm = misc[:, 0:8].view(np.float32)
    sm[:, 0] = np.asarray(inputs["current_time"][sl], np.float32)[:, 0]
    sm[:, 1] = np.asarray(inputs["used_capacity"][sl], np.float32)[:, 0]
    sm[:, 2] = np.asarray(inputs["vehicle_capacity"][sl], np.float32)[:, 0]
    sm[:, 3] = np.asarray(inputs["i"][sl]).astype(np.float32)[:, 0]
    si = misc[:, 8:16].view(np.int32)
    si[:, 0] = nat_row(cur)
    si[:, 1] = nat_row(fn)
    misc[:, 16:48] = consts["_negm"].view(np.uint16)
    ttm = np.asarray(inputs["travel_time_matrix"], np.float32)
    rbf = (ttm[h3[bb, cur]] * C_TRAVEL).astype(NBF)
    blob[R0_RBF : R0_RBF + 512] = rbf.view(np.uint16).reshape(512, 128)
    return {"blob": blob}


_last_results = None


def kernel(**inputs):
    global _last_results
    nc = _cached_program()
    consts = _consts()
    embb_all = np.asarray(inputs["node_emb"], dtype=np.float32).astype(NBF)
    with ThreadPoolExecutor(NCORES) as ex:
        in_maps = list(ex.map(
            lambda c: make_in_map(inputs, c, consts, embb_all), range(NCORES)))
    import os
    trace = bool(int(os.environ.get("KERNEL_TRACE", "0")))
    rr = run_bass_kernel_spmd(nc, in_maps, list(range(NCORES)), trace=trace)
    _last_results = rr
    out = np.concatenate([np.asarray(rr.results[c]["out"]).astype(np.float32)
                          for c in range(NCORES)], axis=0)
    return out


# revision 9
# speedup vs baseline: 1.3005x; 1.3005x over previous
"""Trainium2 Bass kernel for nn_DARPDecoder (sparse_attention).

Pure data parallel over batch: 8 cores x 128 batches. Per batch:
  score[b,n] = emb[b,n,:].qk[b] - C*T[cur_h3[b], h3[b,n]]; tanh-clip, mask,
  log_softmax, where qk[b] = sum_i (W_i @ W_key^T/sqrt(D))^T h_i[b] — W_key and
  the 1/sqrt(D) are folded into the five projection weights on the host, which
  removes the [B,N,D] K tensor and a whole matmul stage.

The end-to-end metric is dominated by host->device input transfer over the
axon tunnel (~50 MB/s, uncompressible data), so the kernel is organized to
minimize bytes shipped per core (~17.3 MB, vs 35.5 MB for the two-layout
variant):
  - node_emb ships ONCE per core, bf16, in the natural [n,d] chunk layout.
    The [d,n] tiles needed by pass 2 are produced on device by PE transposes
    (32 x [128,128] per chunk) into a rotating PSUM pool, copied to SBUF by
    DVE. PE/DVE time is microseconds; transfer savings are seconds.
  - travel_time_matrix is not shipped; the host gathers the 128 needed rows
    (T[cur_h3[b], :]) and pre-scales by C_TRAVEL into a bf16 [128,512] input.
  - the 16 psel selection matrices (-1 diagonals on rows {16g+k}) are built
    on device from the identity and a [128,16] mask table (4 KB) instead of
    shipping 512 KB of constants.
  - visited/action_mask ship as one u8 [128,512] with 2 bits packed
    (vis + 2*act), unpacked by 4 DVE ops.

Device structure (unchanged math from the validated two-layout kernel):
  - pass 1: lhsT = nat quarter [128n,128d], rhs = [ones/512 | vf/cnt] 2 cols
    -> graph/visited sums, PSUM [128d, 2 cols/batch], pool-rotated tiles.
  - pass 2: lhsT = transposed quarter [128d,128n], rhs = qk[:,b] single col
    -> score quarters, copied per-chunk to SBUF, transposed back to
    [128b, 512n] by 4 TensorE transposes at the tail.
  - travel bias: per-batch gpsimd indirect_copy from the host-gathered rows
    (each 16-partition group shares one index stream; psel[k] keeps exactly
    rows {16g+k}), accumulated into one PSUM bank during the stream.
  - tail: tanh -> exp(10*tanh) on Act, masked softmax-sum via Pool-multiply +
    DVE-reduce; log_softmax needs no max shift (tanh clips |s| to 10).

Known-invalid-on-HW constructs (sim passes, device faults) to avoid:
  tensor_tensor_reduce; concurrent PE-write + DVE-read on one PSUM bank;
  two PSUM operands in one DVE op.
"""

import functools
import math

import numpy as np
import ml_dtypes

import concourse.bass as bass
import concourse.mybir as mybir
import concourse.tile as tile
from concourse import bacc
from concourse.bass_utils import run_bass_kernel_spmd

BF16 = mybir.dt.bfloat16
F32 = mybir.dt.float32
I32 = mybir.dt.int32
U16 = mybir.dt.uint16
U8 = mybir.dt.uint8
Alu = mybir.AluOpType
AF = mybir.ActivationFunctionType
AX = mybir.AxisListType

B, N, D, NCORES = 1024, 512, 128, 8
BC = B // NCORES  # 128 batches/core
NCH, CHB = 16, 8  # 16 stream chunks x 8 batches
MAX_TIME = 1440.0
TANH_CLIP = 10.0
C_TRAVEL = 1.0 / MAX_TIME / math.sqrt(2.0)
INV_SQRT_D = 1.0 / math.sqrt(D)
NBF = np.dtype(ml_dtypes.bfloat16)

# consts layout: [idn 128 | wl 128 | wf 128 | wg 128 | wv 128 | ws 128]
CW_COLS = 128 + 5 * 128

# single packed input blob: one u16 [R0_TOT, 128] tensor per core. Each
# region's logical [128, X]-shaped view occupies X//128 consecutive DRAM rows
# per logical row (plain row-major reshape), bitcast to its real dtype.
R0_EMB = 0                       # [65536, 128] bf16 (nat chunk layout)
R0_H3W = R0_EMB + NCH * 128 * 32  # [128, 512] u16
R0_RBF = R0_H3W + 512            # [128, 512] bf16
R0_VAM = R0_RBF + 512            # [128, 512] u8
R0_CW = R0_VAM + 256             # [128, 768] bf16
R0_MISC = R0_CW + 768            # [128, 128] u16: smallf f32 | smalli i32 | negm f32
R0_TOT = R0_MISC + 128

# queue for each streamed nat chunk. gpsimd (Pool software-DGE) takes the
# first 4 back-to-back (its indirect_copy work fills the later window);
# sync/scalar alternate over the rest.
NAT_Q = ["gpsimd"] * 4 + ["sync", "scalar"] * 6


def _emit(nc, tc, T):
    blob = T["blob"].ap()
    ap = {
        "emb_nat_t": blob[R0_EMB : R0_EMB + 65536, :].bitcast(BF16),
        "h3w": blob[R0_H3W : R0_H3W + 512, :].rearrange(
            "(p f) c -> p (f c)", f=4),
        "rbf": blob[R0_RBF : R0_RBF + 512, :].bitcast(BF16).rearrange(
            "(p f) c -> p (f c)", f=4),
        "vam": blob[R0_VAM : R0_VAM + 256, :].bitcast(U8).rearrange(
            "(p t) c -> p (t c)", t=2),
        "constsW": blob[R0_CW : R0_CW + 768, :].bitcast(BF16).rearrange(
            "(p s) c -> p (s c)", s=6),
        "smallf": blob[R0_MISC : R0_MISC + 128, 0:8].bitcast(F32),
        "smalli": blob[R0_MISC : R0_MISC + 128, 8:16].bitcast(I32),
        "negm": blob[R0_MISC : R0_MISC + 128, 16:48].bitcast(F32),
        "out": T["out"].ap(),
    }
    eng = {"sync": nc.sync, "scalar": nc.scalar, "gpsimd": nc.gpsimd}

    with (
        tc.tile_pool(name="cp", bufs=1) as cp,
        tc.tile_pool(name="stn", bufs=6) as stn,
        tc.tile_pool(name="ste", bufs=3) as ste,
        tc.tile_pool(name="wk", bufs=2) as wk,
        tc.tile_pool(name="ps_s", bufs=2, space="PSUM") as ps_s,
        tc.tile_pool(name="ps_q", bufs=2, space="PSUM") as ps_q,
        tc.tile_pool(name="ps_tv", bufs=1, space="PSUM") as ps_tv,
        tc.tile_pool(name="ps_e", bufs=2, space="PSUM") as ps_e,
        tc.tile_pool(name="ps_qk", bufs=1, space="PSUM") as ps_qk,
    ):
        # ---------- consolidated small loads ----------
        # scalar queue: consts first (idn gates all prologue transposes),
        # then the travel rows and packed masks, then its nat tiles.
        cw = cp.tile([128, CW_COLS], BF16, name="constsW")
        nc.scalar.dma_start(out=cw[:], in_=ap["constsW"])
        rbf = cp.tile([BC, N], BF16, name="rbf")
        nc.scalar.dma_start(out=rbf[:], in_=ap["rbf"])
        vam = cp.tile([BC, N], U8, name="vam")
        nc.scalar.dma_start(out=vam[:], in_=ap["vam"])
        smf = cp.tile([128, 4], F32, name="smallf")
        nc.sync.dma_start(out=smf[:], in_=ap["smallf"])
        smi = cp.tile([128, 4], I32, name="smalli")
        nc.sync.dma_start(out=smi[:], in_=ap["smalli"])
        gcur = cp.tile([BC, 1], I32, name="gcur")
        nc.vector.tensor_copy(out=gcur[:], in_=smi[:, 0:1])
        gfn = cp.tile([BC, 1], I32, name="gfn")
        nc.vector.tensor_copy(out=gfn[:], in_=smi[:, 1:2])
        gcur, gfn = gcur[:], gfn[:]
        h3w = cp.tile([128, 512], U16, name="h3w")
        nc.gpsimd.dma_start(out=h3w[:], in_=ap["h3w"])

        idn = cw[:, 0:128]
        wb = 128
        wl, wf, wg, wv = (cw[:, wb + 128 * i : wb + 128 * (i + 1)] for i in range(4))
        ws = cw[0:4, wb + 512 : wb + 640]

        # ---------- activation-table preload (tanh+exp share one table) ----------
        dum = cp.tile([1, 1], F32, name="dum")
        nc.vector.memset(dum[:], 1.0)
        dmo = wk.tile([1, 1], F32, tag="dmo")
        nc.scalar.activation(out=dmo[:], in_=dum[:], func=AF.Tanh, scale=1.0)

        idnf = cp.tile([128, 128], F32, name="idnf")
        nc.vector.tensor_copy(out=idnf[:], in_=idn)

        # psel[k] = idn * negm[:,k] (per-partition broadcast): -1 diagonal on
        # rows {16g+k}, zero elsewhere. (scalar1 APs must be f32.)
        negmf = cp.tile([128, 16], F32, name="negmf")
        nc.sync.dma_start(out=negmf[:], in_=ap["negm"])
        pselt = cp.tile([128, 16 * 128], BF16, name="pselt")
        for k in range(16):
            nc.vector.tensor_scalar(
                out=pselt[:, 128 * k : 128 * (k + 1)], in0=idn,
                scalar1=negmf[:, k : k + 1], scalar2=None, op0=Alu.mult)
        psel = [pselt[:, 128 * k : 128 * (k + 1)] for k in range(16)]

        # ---------- gathers (gpsimd queue; need smalli only) ----------
        hc_rows = cp.tile([BC, D], BF16, name="hc_rows")
        nc.gpsimd.indirect_dma_start(
            out=hc_rows[:], out_offset=None, in_=ap["emb_nat_t"],
            in_offset=bass.IndirectOffsetOnAxis(ap=gcur, axis=0))
        hf_rows = cp.tile([BC, D], BF16, name="hf_rows")
        nc.gpsimd.indirect_dma_start(
            out=hf_rows[:], out_offset=None, in_=ap["emb_nat_t"],
            in_offset=bass.IndirectOffsetOnAxis(ap=gfn, axis=0))

        # ---------- masks / counts (vam = visited + 2*action_mask) ----------
        vamf = cp.tile([BC, N], F32, name="vamf")
        nc.vector.tensor_copy(out=vamf[:], in_=vam[:])
        amf = cp.tile([BC, N], F32, name="amf")
        nc.vector.tensor_scalar(out=amf[:], in0=vamf[:], scalar1=2.0,
                                scalar2=None, op0=Alu.is_ge)
        amn = cp.tile([BC, N], F32, name="amn")
        nc.vector.tensor_scalar_mul(out=amn[:], in0=amf[:], scalar1=-2.0)
        visf = cp.tile([BC, N], F32, name="visf")
        nc.vector.tensor_add(out=visf[:], in0=vamf[:], in1=amn[:])
        vc = cp.tile([BC, 1], F32, name="vc")
        nc.vector.tensor_reduce(out=vc[:], in_=visf[:], axis=AX.X, op=Alu.add)
        nc.vector.tensor_scalar_max(out=vc[:], in0=vc[:], scalar1=1.0)
        vcr = cp.tile([BC, 1], F32, name="vcr")
        nc.vector.reciprocal(out=vcr[:], in_=vc[:])
        vsc = cp.tile([BC, N], BF16, name="vsc")
        nc.vector.tensor_scalar(out=vsc[:], in0=visf[:], scalar1=vcr[:, :1],
                                scalar2=None, op0=Alu.mult)

        # vs2[q]: [128 n_q, 2*BC] cols (2b, 2b+1) = (1/512, vf[b, n_q]/cnt_b)
        vs2 = []
        for q in range(4):
            v = cp.tile([128, 2 * BC], BF16, name=f"vs2_{q}")
            nc.vector.memset(v[:].rearrange("p (b two) -> p b two", two=2)[:, :, 0:1],
                             1.0 / N)
            pt = ps_e.tile([128, 512], BF16, tag="et_ps")
            nc.tensor.transpose(out=pt[:, 0:128], in_=vsc[:, 128 * q : 128 * (q + 1)],
                                identity=idn)
            nc.vector.tensor_copy(
                out=v[:].rearrange("p (b two) -> p b two", two=2)[:, :, 1:2],
                in_=pt[:, 0:128].rearrange("p (b one) -> p b one", one=1))
            vs2.append(v)

        # ---------- masks for the epilogue (hoisted off the tail) ----------
        m10 = cp.tile([BC, N], F32, name="m10")
        nc.vector.tensor_scalar_mul(out=m10[:], in0=amf[:], scalar1=TANH_CLIP)
        m2 = cp.tile([BC, N], F32, name="m2")
        nc.vector.tensor_scalar(out=m2[:], in0=amf[:], scalar1=1.0, scalar2=1e8,
                                op0=Alu.subtract, op1=Alu.mult)

        # ---------- h_cur/h_first/state transposes ----------
        hct = cp.tile([128, BC], BF16, name="hct")
        pt1 = ps_e.tile([128, 512], BF16, tag="et_ps")
        nc.tensor.transpose(out=pt1[:, 0:128], in_=hc_rows[:], identity=idn)
        nc.vector.tensor_copy(out=hct[:], in_=pt1[:, 0:128])
        hft = cp.tile([128, BC], BF16, name="hft")
        pt2 = ps_e.tile([128, 512], BF16, tag="et_ps")
        nc.tensor.transpose(out=pt2[:, 0:128], in_=hf_rows[:], identity=idn)
        nc.vector.tensor_copy(out=hft[:], in_=pt2[:, 0:128])

        sf = cp.tile([BC, 4], F32, name="sf")
        nc.vector.tensor_sub(out=sf[:, 0:1], in0=smf[:, 2:3], in1=smf[:, 1:2])
        nc.vector.tensor_scalar_mul(out=sf[:, 1:2], in0=smf[:, 0:1], scalar1=1.0 / MAX_TIME)
        nc.vector.tensor_scalar_mul(out=sf[:, 2:3], in0=smf[:, 3:4], scalar1=1.0 / (2.0 * N))
        nc.vector.memset(sf[:, 3:4], 1.0)
        sfb = cp.tile([BC, 4], BF16, name="sfb")
        nc.vector.tensor_copy(out=sfb[:], in_=sf[:])
        pt3 = ps_e.tile([128, 512], BF16, tag="et_ps")
        nc.tensor.transpose(out=pt3[:4, 0:128], in_=sfb[:], identity=idn)
        sft = cp.tile([4, BC], BF16, name="sft")
        nc.vector.tensor_copy(out=sft[:], in_=pt3[:4, :BC])

        # ---------- persistent accumulators ----------
        gvb = cp.tile([128, 2 * BC], BF16, name="gvb")
        qk = cp.tile([128, BC], BF16, name="qk")
        scA = cp.tile([128, N], F32, name="scA")           # scores [n_q, (q,b)]
        pvT = ps_tv.tile([128, N], F32, tag="trav")        # -C*travel (PE-only)

        nat_src = ap["emb_nat_t"].rearrange("(k p blk) d -> k p (blk d)",
                                            k=NCH, p=128, blk=32)

        # travel steps spread over stream iterations: gather gk[t] rows
        # {16g+t} = C*T[cur_h3[b], h3[b,:]] straight from rbf (rows of other
        # batches in each 16-partition group are garbage; psel[t] keeps only
        # row 16g+t), then accumulate -travel into pvT.
        ic_iter = [1 + (k * 12) // 16 for k in range(16)]
        mm_iter = [min(NCH - 2, i + 1) for i in ic_iter]
        gk = [None] * 16

        # ---------- streamed chunks ----------
        for k in range(NCH):
            nat = stn.tile([128, 4096], BF16, tag="nat")
            eng[NAT_Q[k]].dma_start(out=nat[:], in_=nat_src[k])

            for t in [i for i, it in enumerate(ic_iter) if it == k]:
                g = cp.tile([128, N], BF16, name=f"gk{t}")
                nc.gpsimd.indirect_copy(out=g[:], data=rbf[:],
                                        idxs=h3w[:, 32 * t : 32 * (t + 1)],
                                        i_know_ap_gather_is_preferred=True)
                gk[t] = g
            for t in [i for i, it in enumerate(mm_iter) if it == k]:
                nc.tensor.matmul(out=pvT[:], lhsT=psel[t], rhs=gk[t][:],
                                 start=(t == 0), stop=True, skip_group_check=True)
            if k == NCH - 1:
                tvs = cp.tile([BC, N], F32, name="tvs")
                nc.vector.tensor_copy(out=tvs[:], in_=pvT[:])

            # build the [d,n] chunk by PE transposes of the 32 nat blocks
            et = ste.tile([128, 4096], BF16, tag="et")
            for j in range(CHB):
                pe = ps_e.tile([128, 512], BF16, tag="et_ps")
                for q in range(4):
                    nc.tensor.transpose(
                        out=pe[:, 128 * q : 128 * (q + 1)],
                        in_=nat[:, (j * 4 + q) * 128 : (j * 4 + q + 1) * 128],
                        identity=idn)
                nc.vector.tensor_copy(out=et[:, j * 512 : (j + 1) * 512], in_=pe[:])

            # pass 1: graph/visited sums, batch j -> pSk[:, 2j:2j+2]
            pSk = ps_s.tile([128, 2 * CHB], F32, tag="sums")
            for j in range(CHB):
                b = k * CHB + j
                for q in range(4):
                    nc.tensor.matmul(
                        out=pSk[:, 2 * j : 2 * j + 2],
                        lhsT=nat[:, (j * 4 + q) * 128 : (j * 4 + q + 1) * 128],
                        rhs=vs2[q][:, 2 * b : 2 * b + 2],
                        start=(q == 0), stop=(q == 3), skip_group_check=True)
            nc.vector.tensor_copy(out=gvb[:, 16 * k : 16 * (k + 1)], in_=pSk[:])

            # q/qk for this chunk's 8 batches
            sl = slice(8 * k, 8 * (k + 1))
            g_sl = gvb[:, 16 * k : 16 * (k + 1)].rearrange("p (b two) -> p b two", two=2)
            psq = ps_qk.tile([128, 8], F32, tag="psq")
            nc.tensor.matmul(out=psq[:], lhsT=wl, rhs=hct[:, sl], start=True, stop=True)
            nc.tensor.matmul(out=psq[:], lhsT=wf, rhs=hft[:, sl], start=False, stop=True,
                             skip_group_check=True)
            nc.tensor.matmul(out=psq[:], lhsT=wg, rhs=g_sl[:, :, 0:1], start=False,
                             stop=True, skip_group_check=True)
            nc.tensor.matmul(out=psq[:], lhsT=wv, rhs=g_sl[:, :, 1:2], start=False,
                             stop=True, skip_group_check=True)
            nc.tensor.matmul(out=psq[:], lhsT=ws, rhs=sft[:, sl], start=False, stop=True,
                             skip_group_check=True)
            nc.vector.tensor_copy(out=qk[:, sl], in_=psq[:])

            # pass 2: score quarters, batch j -> pqk[:, 4j+q]; copy to the
            # quarter-major SBUF tile right away (keeps the tail short)
            pqk = ps_q.tile([128, 4 * CHB], F32, tag="scT")
            for j in range(CHB):
                b = k * CHB + j
                for q in range(4):
                    nc.tensor.matmul(
                        out=pqk[:, 4 * j + q : 4 * j + q + 1],
                        lhsT=et[:, j * 512 + 128 * q : j * 512 + 128 * (q + 1)],
                        rhs=qk[:, b : b + 1],
                        start=True, stop=True, skip_group_check=True)
            nc.vector.tensor_copy(
                out=scA[:].rearrange("p (q b) -> p q b", q=4)[:, :, 8 * k : 8 * (k + 1)],
                in_=pqk[:].rearrange("p (b q) -> p q b", q=4))

        # ---------- tail: per-half pipeline across PE/Act/DVE ----------
        # scA holds all scores [n_q, 4b+q]; transpose quarter q via stride-4
        # lhsT view, accumulating onto -travel in pvT. Then per half:
        # th = tanh(s/10) (Act), exm = exp(10*th) (Act, no mask needed first),
        # se = sum(exm*am) fused on DVE, msk for the output in parallel.
        # log_softmax has no max shift (tanh clips |s| to 10).
        msk = cp.tile([BC, N], F32, name="msk")
        seq = cp.tile([BC, 2], F32, name="seq")
        # transpose the four score quarters into pvT's bank (free after the tvs
        # copy; all 4 PE writes precede the single DVE read -> no bank overlap)
        ssb = cp.tile([BC, N], F32, name="ssb")
        for qq in range(4):
            qb = slice(128 * qq, 128 * (qq + 1))
            nc.tensor.transpose(out=pvT[:, qb], in_=scA[:, qb], identity=idnf[:])
        nc.vector.tensor_add(out=ssb[:], in0=pvT[:], in1=tvs[:])
        for h in range(2):
            blk = slice(256 * h, 256 * (h + 1))
            thq = wk.tile([128, 256], F32, tag="thq")
            nc.scalar.activation(out=thq[:], in_=ssb[:, blk], func=AF.Tanh,
                                 scale=1.0 / TANH_CLIP)
            exq = wk.tile([128, 256], F32, tag="exq")
            nc.scalar.activation(out=exq[:], in_=thq[:], func=AF.Exp,
                                 scale=TANH_CLIP)
            exm = wk.tile([128, 256], F32, tag="exm")
            nc.gpsimd.tensor_mul(out=exm[:], in0=exq[:], in1=amf[:, blk])
            nc.vector.tensor_reduce(out=seq[:, h : h + 1], in_=exm[:], axis=AX.X,
                                    op=Alu.add)
            nc.gpsimd.tensor_mul(out=msk[:, blk], in0=thq[:], in1=m10[:, blk])
            nc.gpsimd.tensor_add(out=msk[:, blk], in0=msk[:, blk], in1=m2[:, blk])
        se = cp.tile([BC, 1], F32, name="se")
        nc.gpsimd.tensor_add(out=se[:], in0=seq[:, 0:1], in1=seq[:, 1:2])
        lse = cp.tile([BC, 1], F32, name="lse")
        nc.scalar.activation(out=lse[:], in_=se[:], func=AF.Ln)
        fin = wk.tile([BC, N], BF16, tag="fin")
        for h, ve, de in ((0, nc.vector, nc.sync), (1, nc.gpsimd, nc.scalar)):
            blk = slice(256 * h, 256 * (h + 1))
            ve.tensor_scalar(out=fin[:, blk], in0=msk[:, blk],
                             scalar1=lse[:, :1], scalar2=None, op0=Alu.subtract)
            de.dma_start(out=ap["out"][:, blk], in_=fin[:, blk])


def build_program():
    nc = bacc.Bacc("TRN2", target_bir_lowering=False, debug=False)
    dt = nc.dram_tensor
    T = {}

    def din(name, shape, dtype):
        T[name] = dt(name, shape, dtype, kind="ExternalInput")

    din("blob", [R0_TOT, 128], U16)  # all inputs, packed (see R0_* layout)
    T["out"] = dt("out", [BC, N], BF16, kind="ExternalOutput")

    with tile.TileContext(nc) as tc:
        _emit(nc, tc, T)
    nc.compile()
    return nc


@functools.cache
def _cached_program():
    return build_program()


def _consts():
    negm = np.zeros((128, 16), np.float32)
    p = np.arange(128)
    for k in range(16):
        negm[p % 16 == k, k] = -1.0
    return {"_idn": np.eye(128, dtype=NBF), "_negm": negm}


def make_in_map(inputs, core, consts=None, embb_all=None):
    """Host-side shard + relayout for one core (pure layout/dtype work)."""
    sl = slice(BC * core, BC * (core + 1))
    if embb_all is not None:
        embb = embb_all[sl]
    else:
        embb = np.asarray(inputs["node_emb"][sl], dtype=np.float32).astype(NBF)
    consts = consts or _consts()
    blob = np.zeros((R0_TOT, 128), np.uint16)
    # emb in nat chunk layout, relayouted straight into the blob
    np.copyto(blob[R0_EMB : R0_EMB + 65536].view(NBF).reshape(NCH, 128, CHB, 4, D),
              embb.reshape(NCH, CHB, 4, 128, D).transpose(0, 3, 1, 2, 4))
    h3 = np.asarray(inputs["h3_indices"][sl]).astype(np.int32)
    h3w = np.ascontiguousarray(
        h3.reshape(8, 16, 32, 16).transpose(1, 0, 3, 2).reshape(16, 128, 32)
        .transpose(1, 0, 2)).reshape(128, 512).astype(np.uint16)
    blob[R0_H3W : R0_H3W + 512] = h3w.reshape(512, 128)
    vam = (np.asarray(inputs["visited"][sl]).astype(np.uint8)
           + 2 * np.asarray(inputs["action_mask"][sl]).astype(np.uint8))
    blob[R0_VAM : R0_VAM + 256] = vam.view(np.uint16).reshape(256, 128)
    wkT = np.asarray(inputs["W_key"], np.float32).T * INV_SQRT_D
    w = np.zeros((128, 640), np.float32)
    w[:, 0:128] = np.asarray(inputs["W_last"], np.float32) @ wkT
    w[:, 128:256] = np.asarray(inputs["W_first"], np.float32) @ wkT
    w[:, 256:384] = np.asarray(inputs["W_graph"], np.float32) @ wkT
    w[:, 384:512] = np.asarray(inputs["W_visited"], np.float32) @ wkT
    w[0:3, 512:640] = np.asarray(inputs["W_state"], np.float32) @ wkT
    w[3, 512:640] = np.asarray(inputs["b_state"], np.float32) @ wkT
    cwv = blob[R0_CW : R0_CW + 768].view(NBF).reshape(128, 768)
    cwv[:, 0:128] = consts["_idn"]
    cwv[:, 128:768] = w.astype(NBF)
    cur = np.asarray(inputs["current_node"][sl]).astype(np.int64)[:, 0]
    prv = np.asarray(inputs["previous_action"][sl]).astype(np.int64)[:, 0]
    fst = np.asarray(inputs["first_node"][sl]).astype(np.int64)
    fn = np.where((prv == 0) & (cur != 0), cur, fst)
    fn = np.where(cur == 0, 0, fn)
    bb = np.arange(BC)

    def nat_row(idx):
        # row of emb_nat_t [(k, n%128, (j,q))] holding emb[b, idx, :]
        return (bb // 8) * 4096 + (idx % 128) * 32 + (bb % 8) * 4 + idx // 128

    misc = blob[R0_MISC : R0_MISC + 128]
    sm = misc[:, 0:8].view(np.float32)
    sm[:, 0] = np.asarray(inputs["current_time"][sl], np.float32)[:, 0]
    sm[:, 1] = np.asarray(inputs["used_capacity"][sl], np.float32)[:, 0]
    sm[:, 2] = np.asarray(inputs["vehicle_capacity"][sl], np.float32)[:, 0]
    sm[:, 3] = np.asarray(inputs["i"][sl]).astype(np.float32)[:, 0]
    si = misc[:, 8:16].view(np.int32)
    si[:, 0] = nat_row(cur)
    si[:, 1] = nat_row(fn)
    misc[:, 16:48] = consts["_negm"].view(np.uint16)
    ttm = np.asarray(inputs["travel_time_matrix"], np.float32)
    rbf = (ttm[h3[bb, cur]] * C_TRAVEL).astype(NBF)
    blob[R0_RBF : R0_RBF + 512] = rbf.view(np.uint16).reshape(512, 128)
    return {"blob": blob}


_last_results = None


def kernel(**inputs):
    global _last_results
    nc = _cached_program()
    consts = _consts()
    in_maps = [make_in_map(inputs, c, consts) for c in range(NCORES)]
    import os
    trace = bool(int(os.environ.get("KERNEL_TRACE", "0")))
    rr = run_bass_kernel_spmd(nc, in_maps, list(range(NCORES)), trace=trace)
    _last_results = rr
    out = np.concatenate([np.asarray(rr.results[c]["out"]).astype(np.float32)
                          for c in range(NCORES)], axis=0)
    return out
